# revision 15
# baseline (speedup 1.0000x reference)
"""GuidedFilter (3-angle iterated boxfilter) on 8 trn2 NeuronCores.

Math: reference iterates  X <- X + (B_a(y) - B_a(X))/N_a  over 3 rotated-line
kernels B_a (17x5; the 0-degree one is 17x1).  With D = y - X this is
D <- D + U_a,  U_a = -B_a(D)/N_a,  X_final = y - D_final.

Device mapping: core (b, h) = (i//4, i%4) handles batch b, rows [512h, +512).
576-row slab (shrink-halo 32/side), 5 row-chunks of 128 (stride 112).
Per (angle, chunk) the update D + U is ONE PSUM accumulation group:
  - slot C: plain bf16 matmul, lhsT = (I - g*W_center) banded, rhs = D (bf16)
  - slots A,B: fp8 DoubleRow pair matmuls, each fusing TWO side taps
    (dx pairs (0,4) and (1,3)) against H = fp8(D), at bf16-slot cost.
PSUM holds D_next.  Evacuation: Act (even chunks) / DVE (odd chunks) copy
PSUM->SBUF bf16 (never touching the same PSUM tensor concurrently); Pool
casts the bf16 tile -> fp8 H tile; DMA syncs 8-row chunk overlaps, DMA out.
g = 1/N(row, center-col) is folded per-output-row into the weights
(3 row variants per angle for image-border rows).  The 4 left/right edge
columns (where N varies per column) are recomputed exactly on the host.
"""

import numpy as np
import ml_dtypes

M_IMG = 2048
N_IMG = 2048
BATCH = 2
H_SHARDS = 4
SH = 512            # rows per shard
SLAB = 576          # shard + 2*32
CW = 2052           # bf16 D tile width: 2 zero-pad cols each side
HCW = 2056          # fp8 H tile width: 4 zero-pad cols each side (aligned writes)
NCHUNK = 5
CH_STEP = 112
KH = 17
PC = 8
PAIRS = [(0, 4), (1, 3)]   # fp8 DoubleRow tap pairs (dx indices) for 17x5 angles
N_WC = 9                   # bf16 center lhsT count: a*3+v
N_WP = 12                  # fp8 pair lhsT count: ap*6 + v*2 + pi


def _host_prep(X, y, kern_in, N_norm):
    kern = np.asarray(kern_in, np.float64)[:, 0]     # (3,17,5)
    N = np.asarray(N_norm, np.float64)[:, 0]         # (3,2048,2048)
    D0 = (np.asarray(y) - np.asarray(X))[:, 0]       # (2,2048,2048) f32

    ginv_full = 1.0 / N[:, :, N_IMG // 2]            # (3,2048)

    in_maps = []
    for core in range(BATCH * H_SHARDS):
        b, h = core // H_SHARDS, core % H_SHARDS
        gs = SH * h - 32                             # global row of slab row 0

        d0s = np.zeros((SLAB, CW), np.float32)
        r0, r1 = max(0, gs), min(M_IMG, gs + SLAB)
        d0s[r0 - gs:r1 - gs, 2:2 + N_IMG] = D0[b, r0:r1]
        d0b = d0s.astype(ml_dtypes.bfloat16)

        # --- banded weight matrices -------------------------------------
        # variant v: 0 -> chunk 0, 1 -> chunks 1..3, 2 -> chunk 4.
        wcs = np.zeros((N_WC, 128, 128), np.float64)       # bf16 center+identity
        wps = np.zeros((N_WP, 128, 2, 128), np.float64)    # fp8 pairs
        for a in range(3):
            for v in range(3):
                c_of_v = {0: 0, 1: 1, 2: 4}[v]
                g_glob = gs + CH_STEP * c_of_v + np.arange(128)
                mask = (g_glob >= 0) & (g_glob < M_IMG)
                gv = np.where(mask, ginv_full[a][np.clip(g_glob, 0, M_IMG - 1)], 0.0)
                Wc = wcs[a * 3 + v]
                for m in range(8, 120):
                    if mask[m]:
                        Wc[m - PC:m - PC + KH, m] -= gv[m] * kern[a, :, 2]
                        Wc[m, m] += 1.0
                if a != 1:
                    ap = 0 if a == 0 else 1
                    for pi, (dxL, dxR) in enumerate(PAIRS):
                        Wp = wps[ap * 6 + v * 2 + pi]
                        for m in range(8, 120):
                            if mask[m]:
                                Wp[m - PC:m - PC + KH, 0, m] -= gv[m] * kern[a, :, dxL]
                                Wp[m - PC:m - PC + KH, 1, m] -= gv[m] * kern[a, :, dxR]
        # prepack in SBUF layout (partition-major, contiguous per partition)
        wcs_p = np.ascontiguousarray(
            wcs.transpose(1, 0, 2).reshape(128, N_WC * 128)).astype(ml_dtypes.bfloat16)
        wps_p = np.ascontiguousarray(
            wps.reshape(N_WP, 128, 256).transpose(1, 0, 2).reshape(128, N_WP * 256)
        ).astype(ml_dtypes.float8_e4m3)
        in_maps.append({"d0b": d0b, "wcs": wcs_p, "wps": wps_p})
    return in_maps


# evac engine by chunk: c<=3 -> Act, c=4 -> DVE.
def _act_idx(a, c):
    """1-based position of evac(a,c) in the Act program order (c<=3)."""
    return 4 * a + c + 1


def _evac_wait(eng, a, c, sact, sdve):
    if c <= 3:
        eng.wait_ge(sact, _act_idx(a, c))
    else:
        eng.wait_ge(sdve, a + 1)


def _build_program():
    import concourse.bass as bass
    from concourse import mybir

    f32 = mybir.dt.float32
    bf16 = mybir.dt.bfloat16
    fp8 = mybir.dt.float8e4
    DR = mybir.MatmulPerfMode.DoubleRow
    nc = bass.Bass("TRN2", target_bir_lowering=False)

    d0b_d = nc.dram_tensor("d0b", [SLAB, CW], bf16, kind="ExternalInput")
    wcs_d = nc.dram_tensor("wcs", [128, N_WC * 128], bf16, kind="ExternalInput")
    wps_d = nc.dram_tensor("wps", [128, N_WP * 256], fp8, kind="ExternalInput")
    xo = nc.dram_tensor("xo", [SH, N_IMG], bf16, kind="ExternalOutput")

    Dt = [[nc.alloc_sbuf_tensor(f"d{p}_{c}", [128, CW], bf16) for c in range(NCHUNK)]
          for p in range(2)]
    Ht = [nc.alloc_sbuf_tensor(f"h0_{c}", [128, HCW], fp8) for c in range(NCHUNK)]
    wcs = nc.alloc_sbuf_tensor("wcss", [128, N_WC * 128], bf16)
    wps = nc.alloc_sbuf_tensor("wpss", [128, N_WP * 256], fp8)
    warm = nc.alloc_sbuf_tensor("warm", [128, 16], bf16)
    warm2 = nc.alloc_sbuf_tensor("warm2", [128, 16], bf16)
    warmr = nc.alloc_sbuf_tensor("warmr", [128, 528], bf16)
    ps = [nc.alloc_psum_tensor(f"ps{i}", [128, N_IMG], f32) for i in range(2)]

    def pad_ap(t):
        return bass.AP(t, 0, [[CW, 128], [CW - 2, 2], [1, 2]])

    def hpad_ap(t):
        return bass.AP(t, 0, [[HCW, 128], [HCW - 4, 2], [1, 4]])

    out_rows = [(0, 32, 120), (88, 8, 120), (200, 8, 120), (312, 8, 120), (424, 8, 96)]
    nfill = [1, 2, 2, 2, 1]

    from contextlib import ExitStack
    with ExitStack() as stack:
        block = stack.enter_context(nc.Block())
        sem = lambda n: stack.enter_context(nc.semaphore(n))
        sldw, spe, sact, sdve, spool, sout, swarm = (
            sem("sldw"), sem("spe"), sem("sact"), sem("sdve"), sem("spool"),
            sem("sout"), sem("swarm"))
        sldc = [sem(f"sldc{c}") for c in range(NCHUNK)]
        sldh = [sem(f"sldh{c}") for c in range(NCHUNK)]
        shf = [[sem(f"shf{t}{c}") for c in range(NCHUNK)] for t in range(2)]

        @block.sync
        def _(sp):
            def load_chunk(c):
                sp.dma_start(out=Dt[0][c][:, :],
                             in_=d0b_d[c * CH_STEP:c * CH_STEP + 128, :]
                             ).then_inc(sldc[c], 16)
            load_chunk(0)
            sp.dma_start(out=wcs[:, :], in_=wcs_d[:, :]).then_inc(sldw, 16)
            sp.dma_start(out=wps[:, :], in_=wps_d[:, :]).then_inc(sldw, 16)
            load_chunk(1)
            load_chunk(2)
            load_chunk(3)
            load_chunk(4)
            # halo fills after transitions t=0 (a0->a1) and t=1 (a1->a2)
            for t in range(2):
                q = (t + 1) % 2
                for c in range(NCHUNK):
                    cm = min(c + 1, NCHUNK - 1)
                    sp.wait_ge(sact, _act_idx(t, min(cm, 3)))
                    if cm >= 4:
                        sp.wait_ge(sdve, t + 1)
                    if t == 1:
                        sp.wait_ge(spool, cm + 1)
                    if c > 0:
                        sp.dma_start(out=Dt[q][c][0:8, :],
                                     in_=Dt[q][c - 1][112:120, :]).then_inc(shf[t][c], 16)
                        if t == 1:
                            sp.dma_start(out=Ht[c][0:8, :],
                                         in_=Ht[c - 1][112:120, :]).then_inc(shf[t][c], 16)
                    if c < NCHUNK - 1:
                        sp.dma_start(out=Dt[q][c][120:128, :],
                                     in_=Dt[q][c + 1][8:16, :]).then_inc(shf[t][c], 16)
                        if t == 1:
                            sp.dma_start(out=Ht[c][120:128, :],
                                         in_=Ht[c + 1][8:16, :]).then_inc(shf[t][c], 16)
            # output DMAs
            for c in range(NCHUNK):
                o, p0, p1 = out_rows[c]
                _evac_wait(sp, 2, c, sact, sdve)
                sp.dma_start(out=xo[o:o + (p1 - p0), :],
                             in_=Dt[1][c][p0:p1, 2:2 + N_IMG]).then_inc(sout, 16)
            sp.wait_ge(sout, 16 * NCHUNK)

        @block.tensor
        def _(pe):
            pe.wait_ge(swarm, 1)
            for i in range(12):
                pe.matmul(ps[0][:, 0:512], lhsT=warmr[:, 0:128],
                          rhs=warmr[:, 0:512], start=True, stop=True,
                          skip_group_check=True)
            for a in range(3):
                p = a % 2
                ap = 0 if a == 0 else 1
                for c in range(NCHUNK):
                    g = NCHUNK * a + c
                    if a == 0:
                        if c == 0:
                            pe.wait_ge(sldw, 32)
                        pe.wait_ge(sldh[c], 1)
                    else:
                        pe.wait_ge(shf[a - 1][c], 16 * (1 if a == 1 else 2) * nfill[c])
                    if g >= 2:
                        _evac_wait(pe, (g - 2) // NCHUNK, (g - 2) % NCHUNK, sact, sdve)
                    v = {0: 0, NCHUNK - 1: 2}.get(c, 1)
                    wc_i = a * 3 + v
                    for nt in range(4):
                        o = nt * 512
                        mm = pe.matmul(ps[g % 2][:, o:o + 512],
                                       lhsT=wcs[:, wc_i * 128:(wc_i + 1) * 128],
                                       rhs=Dt[p][c][:, o + 2:o + 514],
                                       start=True, stop=(a == 1),
                                       skip_group_check=True)
                        if a != 1:
                            for pi, (dxL, dxR) in enumerate(PAIRS):
                                wp_i = ap * 6 + v * 2 + pi
                                mm = pe.matmul(
                                    ps[g % 2][:, o:o + 512],
                                    lhsT=bass.AP(wps, wp_i * 256,
                                                 [[N_WP * 256, 128], [128, 2], [1, 128]]),
                                    rhs=bass.AP(Ht[c], o + dxL + 2,
                                                [[HCW, 128], [dxR - dxL, 2], [1, 512]]),
                                    start=False, stop=(pi == len(PAIRS) - 1),
                                    perf_mode=DR, skip_group_check=True)
                        if nt == 3:
                            mm.then_inc(spe, 1)

        @block.scalar
        def _(act):
            act.copy(out=warm2[:, :], in_=warm[:, :])     # preload act table
            for a in range(3):
                qp = (a + 1) % 2
                for c in range(4):
                    g = NCHUNK * a + c
                    act.wait_ge(spe, g + 1)
                    act.copy(out=Dt[qp][c][:, 2:2 + N_IMG],
                             in_=ps[g % 2][:, :]).then_inc(sact, 1)

        @block.vector
        def _(dve):
            dve.memset(warmr[:, :], 0.0).then_inc(swarm, 1)
            for c in range(NCHUNK):
                dve.memset(hpad_ap(Ht[c]), 0.0)
            for c in range(NCHUNK):
                dve.wait_ge(sldc[c], 16)
                dve.tensor_copy(out=Ht[c][:, 4:4 + N_IMG],
                                in_=Dt[0][c][:, 2:2 + N_IMG]).then_inc(sldh[c], 1)
            dve.wait_ge(spe, 5)
            dve.tensor_copy(out=Dt[1][4][:, 2:2 + N_IMG],
                            in_=ps[4 % 2][:, :]).then_inc(sdve, 1)
            for c in range(4):
                dve.wait_ge(sact, _act_idx(1, c))
                dve.tensor_copy(out=Ht[c][:, 4:4 + N_IMG],
                                in_=Dt[0][c][:, 2:2 + N_IMG]).then_inc(spool, 1)
            dve.wait_ge(spe, 10)
            dve.tensor_copy(out=Dt[0][4][:, 2:2 + N_IMG],
                            in_=ps[9 % 2][:, :]).then_inc(sdve, 1)
            dve.tensor_copy(out=Ht[4][:, 4:4 + N_IMG],
                            in_=Dt[0][4][:, 2:2 + N_IMG]).then_inc(spool, 1)
            dve.wait_ge(spe, 15)
            dve.tensor_copy(out=Dt[1][4][:, 2:2 + N_IMG],
                            in_=ps[14 % 2][:, :]).then_inc(sdve, 1)
    return nc


def _edge_strips(D0, kern, N):
    """Exact D3 on the 4 left / 4 right edge columns (f64 host compute)."""
    outs = []
    for side in range(2):
        W = 10
        if side == 0:
            s = D0[:, :, 0:W].astype(np.float64)
            colof = 0
        else:
            s = D0[:, :, N_IMG - W:].astype(np.float64)
            colof = N_IMG - W
        for a in range(3):
            sp = np.pad(s, ((0, 0), (8, 8), (2, 2)))
            B = np.zeros_like(s)
            for t in range(KH):
                for dx in range(5):
                    w = kern[a, t, dx]
                    if w != 0.0:
                        B += w * sp[:, t:t + M_IMG, dx:dx + W]
            Ncols = N[a, :, colof:colof + W]
            s = s - B / Ncols[None]
        outs.append(s[:, :, 0:4] if side == 0 else s[:, :, W - 4:])
    return outs[0], outs[1]


_LAST = None  # BassKernelResults of the most recent run (for test harness)


def kernel(X, y, kernel, N_norm):
    global _LAST
    from concourse.bass_utils import run_bass_kernel_spmd

    kern = np.asarray(kernel, np.float64)[:, 0]
    N = np.asarray(N_norm, np.float64)[:, 0]
    in_maps = _host_prep(X, y, kernel, N_norm)
    nc = _build_program()
    res = run_bass_kernel_spmd(nc, in_maps, list(range(BATCH * H_SHARDS)))
    _LAST = res

    yf = np.asarray(y)[:, 0].astype(np.float64)
    D3 = np.empty((BATCH, M_IMG, N_IMG), np.float64)
    for core in range(BATCH * H_SHARDS):
        b, h = core // H_SHARDS, core % H_SHARDS
        D3[b, SH * h:SH * h + SH, :] = res.results[core]["xo"].astype(np.float64)

    D0 = (np.asarray(y) - np.asarray(X))[:, 0]
    left, right = _edge_strips(D0, kern, N)
    D3[:, :, 0:4] = left
    D3[:, :, N_IMG - 4:] = right
    out = (yf - D3).astype(np.float32)
    return out[:, None]


# revision 16
# speedup vs baseline: 1.0776x; 1.0776x over previous
"""GuidedFilter (3-angle iterated boxfilter) on 8 trn2 NeuronCores.

Math: reference iterates  X <- X + (B_a(y) - B_a(X))/N_a  over 3 rotated-line
kernels B_a (17x5; the middle one is 17x1, 0 degrees).  With D = y - X this
is D <- (I - G_a B_a) D,  X_final = y - D_final.

The 0-degree step is COMPOSED into the first step:  pass 1 applies
(I - G1 B1)(I - G0 B0) as 5 horizontal taps with 33-tall banded vertical
profiles (exact per-row matrix products, including image-border rows);
pass 2 applies (I - G2 B2) with 17-tall bands.  Per (pass, chunk) the
update is ONE PSUM accumulation group:
  - slot C: plain bf16 matmul, lhsT = center-tap band incl identity, rhs = D
  - slots A,B: fp8 DoubleRow pair matmuls, each fusing TWO side taps
    (dx pairs (0,4) and (1,3)) against H = fp8(D), at bf16-slot cost.
Mapping: core (b, h) = (i//4, i%4) handles batch b, rows [512h, +512) via a
576-row slab split into 6 row-chunks of 128 at starts [0,96,192,288,384,448].
Evacuation: Act (chunks 0-4) / DVE (chunk 5) copy PSUM->SBUF bf16; DVE casts
bf16 tiles -> fp8 H tiles; DMA syncs 16-row chunk overlaps, DMA out.
The 4 left/right edge columns (where N varies per column) are recomputed
exactly on the host.
"""

import numpy as np
import ml_dtypes

M_IMG = 2048
N_IMG = 2048
BATCH = 2
H_SHARDS = 4
SH = 512            # rows per shard
SLAB = 576          # shard + 2*32
CW = 2052           # bf16 D tile width: 2 zero-pad cols each side
HCW = 2056          # fp8 H tile width: 4 zero-pad cols each side (aligned writes)
NCHUNK = 6
STARTS = [0, 96, 192, 288, 384, 448]   # chunk window starts within the slab
KH = 17
PC = 8
PAIRS = [(0, 4), (1, 3)]   # DoubleRow tap pairs (dx indices)
N_WC = 6                   # bf16 center lhsT count: pass*3+v
N_WP = 12                  # fp8 pair lhsT count: pass*6 + v*2 + pi
# pass p valid output rows within a 128-row window:
VLO = [16, 8]
VHI = [112, 120]
# output row mapping: (abs out row, p0, p1) per chunk (pass-2 windows)
OUT_ROWS = [(0, 32, 120), (88, 24, 120), (184, 24, 120), (280, 24, 120),
            (376, 24, 120), (472, 56, 96)]
NFILL = [1, 2, 2, 2, 2, 1]


def _band_matrix(kern, ginv, rows, a, dx, include_id):
    """[128,128] matrix M with M[m, m-8+t] -= g(m)*kern[a,t,dx], plus
    identity if include_id; rows outside the image are fully masked."""
    M = np.zeros((128, 128), np.float64)
    mask = (rows >= 0) & (rows < M_IMG)
    gv = np.where(mask, ginv[a][np.clip(rows, 0, M_IMG - 1)], 0.0)
    for m in range(128):
        if not mask[m]:
            continue
        for t in range(KH):
            k = m - PC + t
            if 0 <= k < 128 and kern[a, t, dx] != 0.0:
                M[m, k] -= gv[m] * kern[a, t, dx]
        if include_id:
            M[m, m] += 1.0
    return M


def _host_prep(X, y, kern_in, N_norm):
    kern = np.asarray(kern_in, np.float64)[:, 0]     # (3,17,5)
    N = np.asarray(N_norm, np.float64)[:, 0]         # (3,2048,2048)
    D0 = (np.asarray(y) - np.asarray(X))[:, 0]       # (2,2048,2048) f32

    ginv = 1.0 / N[:, :, N_IMG // 2]                 # (3,2048)

    in_maps = []
    for core in range(BATCH * H_SHARDS):
        b, h = core // H_SHARDS, core % H_SHARDS
        gs = SH * h - 32                             # global row of slab row 0

        d0s = np.zeros((SLAB, CW), np.float32)
        r0, r1 = max(0, gs), min(M_IMG, gs + SLAB)
        d0s[r0 - gs:r1 - gs, 2:2 + N_IMG] = D0[b, r0:r1]
        d0b = d0s.astype(ml_dtypes.bfloat16)

        # weight variants: v0 -> chunk 0, v1 -> chunks 1..4, v2 -> chunk 5
        wcs = np.zeros((N_WC, 128, 128), np.float64)
        wps = np.zeros((N_WP, 128, 2, 128), np.float64)
        for v, st in enumerate([0, 96, 448]):
            rows = gs + st + np.arange(128)
            # pass 1: (I - G1 B1) @ (per-dx parts of I - G0 B0)
            M1 = _band_matrix(kern, ginv, rows, 1, 2, True)
            P = [M1 @ _band_matrix(kern, ginv, rows, 0, dx, dx == 2)
                 for dx in range(5)]
            for p_ in P:
                p_[:VLO[0]] = 0.0
                p_[VHI[0]:] = 0.0
            # pass 2: angle 2 alone
            A = [_band_matrix(kern, ginv, rows, 2, dx, dx == 2)
                 for dx in range(5)]
            for a_ in A:
                a_[:VLO[1]] = 0.0
                a_[VHI[1]:] = 0.0
            for pa, mats in enumerate((P, A)):
                wcs[pa * 3 + v] = mats[2].T
                for pi, (dxL, dxR) in enumerate(PAIRS):
                    wps[pa * 6 + v * 2 + pi, :, 0, :] = mats[dxL].T
                    wps[pa * 6 + v * 2 + pi, :, 1, :] = mats[dxR].T

        wcs_p = np.ascontiguousarray(
            wcs.transpose(1, 0, 2).reshape(128, N_WC * 128)).astype(ml_dtypes.bfloat16)
        wps_p = np.ascontiguousarray(
            wps.reshape(N_WP, 128, 256).transpose(1, 0, 2).reshape(128, N_WP * 256)
        ).astype(ml_dtypes.float8_e4m3)
        in_maps.append({"d0b": d0b, "wcs": wcs_p, "wps": wps_p})
    return in_maps


# evac engine by chunk: c<=4 -> Act, c=5 -> DVE.
def _act_idx(pa, c):
    return 5 * pa + c + 1


def _evac_wait(eng, pa, c, sact, sdve):
    if c <= 4:
        eng.wait_ge(sact, _act_idx(pa, c))
    else:
        eng.wait_ge(sdve, pa + 1)


# halo geometry: chunk c gets rows [0,16) from chunk c-1 and [112,128) from
# chunk c+1.  src partition start within the neighbour's window:
def _halo_src_lo(c):     # for my rows [0,16): neighbour c-1 partitions
    return STARTS[c] - STARTS[c - 1]                  # 96 or 64
def _halo_src_hi(c):     # for my rows [112,128): neighbour c+1 partitions
    return 112 - (STARTS[c + 1] - STARTS[c])          # 16 or 48


def _build_program():
    import concourse.bass as bass
    from concourse import mybir

    f32 = mybir.dt.float32
    bf16 = mybir.dt.bfloat16
    fp8 = mybir.dt.float8e4
    DR = mybir.MatmulPerfMode.DoubleRow
    nc = bass.Bass("TRN2", target_bir_lowering=False)

    d0b_d = nc.dram_tensor("d0b", [SLAB, CW], bf16, kind="ExternalInput")
    wcs_d = nc.dram_tensor("wcs", [128, N_WC * 128], bf16, kind="ExternalInput")
    wps_d = nc.dram_tensor("wps", [128, N_WP * 256], fp8, kind="ExternalInput")
    xo = nc.dram_tensor("xo", [SH, N_IMG], bf16, kind="ExternalOutput")

    # Dt0: D0 for pass 1, then D3 (pass-2 output); Dt1: D2; Ht: H0 then H2.
    Dt0 = [nc.alloc_sbuf_tensor(f"d0_{c}", [128, CW], bf16) for c in range(NCHUNK)]
    Dt1 = [nc.alloc_sbuf_tensor(f"d1_{c}", [128, CW], bf16) for c in range(NCHUNK)]
    Ht = [nc.alloc_sbuf_tensor(f"h_{c}", [128, HCW], fp8) for c in range(NCHUNK)]
    wcs = nc.alloc_sbuf_tensor("wcss", [128, N_WC * 128], bf16)
    wps = nc.alloc_sbuf_tensor("wpss", [128, N_WP * 256], fp8)
    warmr = nc.alloc_sbuf_tensor("warmr", [128, 528], bf16)
    ps = [nc.alloc_psum_tensor(f"ps{i}", [128, N_IMG], f32) for i in range(2)]

    def hpad_ap(t):
        return bass.AP(t, 0, [[HCW, 128], [HCW - 4, 2], [1, 4]])

    from contextlib import ExitStack
    with ExitStack() as stack:
        block = stack.enter_context(nc.Block())
        sem = lambda n: stack.enter_context(nc.semaphore(n))
        sldw, spe, sact, sdve, spool, sout, swarm = (
            sem("sldw"), sem("spe"), sem("sact"), sem("sdve"), sem("spool"),
            sem("sout"), sem("swarm"))
        sldc = [sem(f"sldc{c}") for c in range(NCHUNK)]
        sldh = [sem(f"sldh{c}") for c in range(NCHUNK)]
        shf = [sem(f"shf{c}") for c in range(NCHUNK)]

        @block.sync
        def _(sp):
            def load_chunk(c):
                sp.dma_start(out=Dt0[c][:, :],
                             in_=d0b_d[STARTS[c]:STARTS[c] + 128, :]
                             ).then_inc(sldc[c], 16)
            load_chunk(0)
            sp.dma_start(out=wcs[:, :], in_=wcs_d[:, :]).then_inc(sldw, 16)
            sp.dma_start(out=wps[:, :], in_=wps_d[:, :]).then_inc(sldw, 16)
            for c in range(1, NCHUNK):
                load_chunk(c)
            # halo fills between pass 1 and pass 2 (D2 + H2 strips)
            for c in range(NCHUNK):
                cm = min(c + 1, NCHUNK - 1)
                sp.wait_ge(sact, _act_idx(0, min(cm, 4)))
                if cm >= 5:
                    sp.wait_ge(sdve, 1)
                sp.wait_ge(spool, cm + 1)
                if c > 0:
                    s = _halo_src_lo(c)
                    sp.dma_start(out=Dt1[c][0:16, :],
                                 in_=Dt1[c - 1][s:s + 16, :]).then_inc(shf[c], 16)
                    sp.dma_start(out=Ht[c][0:16, :],
                                 in_=Ht[c - 1][s:s + 16, :]).then_inc(shf[c], 16)
                if c < NCHUNK - 1:
                    s = _halo_src_hi(c)
                    sp.dma_start(out=Dt1[c][112:128, :],
                                 in_=Dt1[c + 1][s:s + 16, :]).then_inc(shf[c], 16)
                    sp.dma_start(out=Ht[c][112:128, :],
                                 in_=Ht[c + 1][s:s + 16, :]).then_inc(shf[c], 16)
            # output DMAs
            for c in range(NCHUNK):
                o, p0, p1 = OUT_ROWS[c]
                _evac_wait(sp, 1, c, sact, sdve)
                sp.dma_start(out=xo[o:o + (p1 - p0), :],
                             in_=Dt0[c][p0:p1, 2:2 + N_IMG]).then_inc(sout, 16)
            sp.wait_ge(sout, 16 * NCHUNK)

        @block.tensor
        def _(pe):
            pe.wait_ge(swarm, 1)
            for i in range(12):
                pe.matmul(ps[0][:, 0:512], lhsT=warmr[:, 0:128],
                          rhs=warmr[:, 0:512], start=True, stop=True,
                          skip_group_check=True)
            for pa in range(2):
                Din = Dt0 if pa == 0 else Dt1
                for c in range(NCHUNK):
                    g = NCHUNK * pa + c
                    if pa == 0:
                        if c == 0:
                            pe.wait_ge(sldw, 32)
                        pe.wait_ge(sldh[c], 1)
                    else:
                        pe.wait_ge(shf[c], 16 * 2 * NFILL[c])
                    if g >= 2:
                        _evac_wait(pe, (g - 2) // NCHUNK, (g - 2) % NCHUNK,
                                   sact, sdve)
                    v = {0: 0, NCHUNK - 1: 2}.get(c, 1)
                    wc_i = pa * 3 + v
                    for nt in range(4):
                        o = nt * 512
                        pe.matmul(ps[g % 2][:, o:o + 512],
                                  lhsT=wcs[:, wc_i * 128:(wc_i + 1) * 128],
                                  rhs=Din[c][:, o + 2:o + 514],
                                  start=True, stop=False,
                                  skip_group_check=True)
                        for pi, (dxL, dxR) in enumerate(PAIRS):
                            wp_i = pa * 6 + v * 2 + pi
                            mm = pe.matmul(
                                ps[g % 2][:, o:o + 512],
                                lhsT=bass.AP(wps, wp_i * 256,
                                             [[N_WP * 256, 128], [128, 2], [1, 128]]),
                                rhs=bass.AP(Ht[c], o + dxL + 2,
                                            [[HCW, 128], [dxR - dxL, 2], [1, 512]]),
                                start=False, stop=(pi == len(PAIRS) - 1),
                                perf_mode=DR, skip_group_check=True)
                        if nt == 3:
                            mm.then_inc(spe, 1)

        @block.scalar
        def _(act):
            act.copy(out=warmr[:, 516:528], in_=warmr[:, 0:12])  # act table
            for pa in range(2):
                Dout = Dt1 if pa == 0 else Dt0
                for c in range(5):
                    g = NCHUNK * pa + c
                    act.wait_ge(spe, g + 1)
                    act.copy(out=Dout[c][:, 2:2 + N_IMG],
                             in_=ps[g % 2][:, :]).then_inc(sact, 1)

        @block.vector
        def _(dve):
            dve.memset(warmr[:, 0:516], 0.0).then_inc(swarm, 1)
            for c in range(NCHUNK):
                dve.memset(hpad_ap(Ht[c]), 0.0)
            for c in range(NCHUNK):
                dve.wait_ge(sldc[c], 16)
                dve.tensor_copy(out=Ht[c][:, 4:4 + N_IMG],
                                in_=Dt0[c][:, 2:2 + N_IMG]).then_inc(sldh[c], 1)
            # H2 casts (into Ht) + chunk-5 evacs; evac(p1,5) early so pass 2
            # is not head-of-line blocked behind the casts.
            dve.wait_ge(sact, _act_idx(0, 0))
            dve.tensor_copy(out=Ht[0][:, 4:4 + N_IMG],
                            in_=Dt1[0][:, 2:2 + N_IMG]).then_inc(spool, 1)
            dve.wait_ge(sact, _act_idx(0, 1))
            dve.tensor_copy(out=Ht[1][:, 4:4 + N_IMG],
                            in_=Dt1[1][:, 2:2 + N_IMG]).then_inc(spool, 1)
            dve.wait_ge(spe, 6)
            dve.tensor_copy(out=Dt1[5][:, 2:2 + N_IMG],
                            in_=ps[5 % 2][:, :]).then_inc(sdve, 1)
            for c in range(2, 5):
                dve.wait_ge(sact, _act_idx(0, c))
                dve.tensor_copy(out=Ht[c][:, 4:4 + N_IMG],
                                in_=Dt1[c][:, 2:2 + N_IMG]).then_inc(spool, 1)
            dve.tensor_copy(out=Ht[5][:, 4:4 + N_IMG],
                            in_=Dt1[5][:, 2:2 + N_IMG]).then_inc(spool, 1)
            dve.wait_ge(spe, 12)
            dve.tensor_copy(out=Dt0[5][:, 2:2 + N_IMG],
                            in_=ps[11 % 2][:, :]).then_inc(sdve, 1)
    return nc


def _edge_strips(D0, kern, N):
    """Exact D3 on the 4 left / 4 right edge columns (f64 host compute)."""
    outs = []
    for side in range(2):
        W = 10
        if side == 0:
            s = D0[:, :, 0:W].astype(np.float64)
            colof = 0
        else:
            s = D0[:, :, N_IMG - W:].astype(np.float64)
            colof = N_IMG - W
        for a in range(3):
            sp = np.pad(s, ((0, 0), (8, 8), (2, 2)))
            B = np.zeros_like(s)
            for t in range(KH):
                for dx in range(5):
                    w = kern[a, t, dx]
                    if w != 0.0:
                        B += w * sp[:, t:t + M_IMG, dx:dx + W]
            Ncols = N[a, :, colof:colof + W]
            s = s - B / Ncols[None]
        outs.append(s[:, :, 0:4] if side == 0 else s[:, :, W - 4:])
    return outs[0], outs[1]


_LAST = None  # BassKernelResults of the most recent run (for test harness)


def kernel(X, y, kernel, N_norm):
    global _LAST
    from concourse.bass_utils import run_bass_kernel_spmd

    kern = np.asarray(kernel, np.float64)[:, 0]
    N = np.asarray(N_norm, np.float64)[:, 0]
    in_maps = _host_prep(X, y, kernel, N_norm)
    nc = _build_program()
    res = run_bass_kernel_spmd(nc, in_maps, list(range(BATCH * H_SHARDS)))
    _LAST = res

    yf = np.asarray(y)[:, 0].astype(np.float64)
    D3 = np.empty((BATCH, M_IMG, N_IMG), np.float64)
    for core in range(BATCH * H_SHARDS):
        b, h = core // H_SHARDS, core % H_SHARDS
        D3[b, SH * h:SH * h + SH, :] = res.results[core]["xo"].astype(np.float64)

    D0 = (np.asarray(y) - np.asarray(X))[:, 0]
    left, right = _edge_strips(D0, kern, N)
    D3[:, :, 0:4] = left
    D3[:, :, N_IMG - 4:] = right
    out = (yf - D3).astype(np.float32)
    return out[:, None]


# revision 17
# speedup vs baseline: 1.2022x; 1.1157x over previous
"""GuidedFilter (3-angle iterated boxfilter) on 8 trn2 NeuronCores.

Math: reference iterates  X <- X + (B_a(y) - B_a(X))/N_a  over 3 rotated-line
kernels B_a (17x5; the middle one is 17x1, 0 degrees).  With D = y - X this
is D <- (I - G_a B_a) D,  X_final = y - D_final.

The 0-degree step is COMPOSED into the first step:  pass 1 applies
(I - G1 B1)(I - G0 B0) as 5 horizontal taps with 33-tall banded vertical
profiles (exact per-row matrix products, including image-border rows);
pass 2 applies (I - G2 B2) with 17-tall bands.  Per (pass, chunk) the
update is ONE PSUM accumulation group:
  - slot C: plain bf16 matmul, lhsT = center-tap band incl identity, rhs = D
  - slots A,B: fp8 DoubleRow pair matmuls, each fusing TWO side taps
    (dx pairs (0,4) and (1,3)) against H = fp8(D), at bf16-slot cost.
Mapping: core (b, h) = (i//4, i%4) handles batch b, rows [512h, +512) via a
576-row slab split into 6 row-chunks of 128 at starts [0,96,192,288,384,448].
Evacuation: Act (chunks 0-4) / DVE (chunk 5) copy PSUM->SBUF bf16; DVE casts
bf16 tiles -> fp8 H tiles; DMA syncs 16-row chunk overlaps, DMA out.
The 4 left/right edge columns (where N varies per column) are recomputed
exactly on the host.
"""

import numpy as np
import ml_dtypes

M_IMG = 2048
N_IMG = 2048
BATCH = 2
H_SHARDS = 4
SH = 512            # rows per shard
SLAB = 576          # shard + 2*32
CW = 2052           # bf16 D tile width: 2 zero-pad cols each side
HCW = 2056          # fp8 H tile width: 4 zero-pad cols each side (aligned writes)
NCHUNK = 6
STARTS = [0, 96, 192, 288, 384, 448]   # chunk window starts within the slab
KH = 17
PC = 8
PAIRS = [(0, 4), (1, 3)]   # DoubleRow tap pairs (dx indices)
N_WC = 6                   # bf16 center lhsT count: pass*3+v
N_WP = 12                  # fp8 pair lhsT count: pass*6 + v*2 + pi
# pass p valid output rows within a 128-row window:
VLO = [16, 8]
VHI = [112, 120]
# output row mapping: (abs out row, p0, p1) per chunk (pass-2 windows)
OUT_ROWS = [(0, 32, 120), (88, 24, 120), (184, 24, 120), (280, 24, 120),
            (376, 24, 120), (472, 56, 96)]
NFILL = [1, 2, 2, 2, 2, 1]


def _band_matrix(kern, ginv, rows, a, dx, include_id):
    """[128,128] matrix M with M[m, m-8+t] -= g(m)*kern[a,t,dx], plus
    identity if include_id; rows outside the image are fully masked."""
    M = np.zeros((128, 128), np.float64)
    mask = (rows >= 0) & (rows < M_IMG)
    gv = np.where(mask, ginv[a][np.clip(rows, 0, M_IMG - 1)], 0.0)
    for m in range(128):
        if not mask[m]:
            continue
        for t in range(KH):
            k = m - PC + t
            if 0 <= k < 128 and kern[a, t, dx] != 0.0:
                M[m, k] -= gv[m] * kern[a, t, dx]
        if include_id:
            M[m, m] += 1.0
    return M


def _host_prep(X, y, kern_in, N_norm):
    kern = np.asarray(kern_in, np.float64)[:, 0]     # (3,17,5)
    N = np.asarray(N_norm, np.float64)[:, 0]         # (3,2048,2048)
    D0 = (np.asarray(y) - np.asarray(X))[:, 0]       # (2,2048,2048) f32

    ginv = 1.0 / N[:, :, N_IMG // 2]                 # (3,2048)

    in_maps = []
    for core in range(BATCH * H_SHARDS):
        b, h = core // H_SHARDS, core % H_SHARDS
        gs = SH * h - 32                             # global row of slab row 0

        d0s = np.zeros((SLAB, CW), np.float32)
        r0, r1 = max(0, gs), min(M_IMG, gs + SLAB)
        d0s[r0 - gs:r1 - gs, 2:2 + N_IMG] = D0[b, r0:r1]
        d0b = d0s.astype(ml_dtypes.bfloat16)

        # weight variants: v0 -> chunk 0, v1 -> chunks 1..4, v2 -> chunk 5
        wcs = np.zeros((N_WC, 128, 128), np.float64)
        wps = np.zeros((N_WP, 128, 2, 128), np.float64)
        for v, st in enumerate([0, 96, 448]):
            rows = gs + st + np.arange(128)
            # pass 1: (I - G1 B1) @ (per-dx parts of I - G0 B0)
            M1 = _band_matrix(kern, ginv, rows, 1, 2, True)
            P = [M1 @ _band_matrix(kern, ginv, rows, 0, dx, dx == 2)
                 for dx in range(5)]
            for p_ in P:
                p_[:VLO[0]] = 0.0
                p_[VHI[0]:] = 0.0
            # pass 2: angle 2 alone
            A = [_band_matrix(kern, ginv, rows, 2, dx, dx == 2)
                 for dx in range(5)]
            for a_ in A:
                a_[:VLO[1]] = 0.0
                a_[VHI[1]:] = 0.0
            for pa, mats in enumerate((P, A)):
                wcs[pa * 3 + v] = mats[2].T
                for pi, (dxL, dxR) in enumerate(PAIRS):
                    wps[pa * 6 + v * 2 + pi, :, 0, :] = mats[dxL].T
                    wps[pa * 6 + v * 2 + pi, :, 1, :] = mats[dxR].T

        wcs_p = np.ascontiguousarray(
            wcs.transpose(1, 0, 2).reshape(128, N_WC * 128)).astype(ml_dtypes.bfloat16)
        wps_p = np.ascontiguousarray(
            wps.reshape(N_WP, 128, 256).transpose(1, 0, 2).reshape(128, N_WP * 256)
        ).astype(ml_dtypes.float8_e4m3)
        in_maps.append({"d0b": d0b, "wcs": wcs_p, "wps": wps_p})
    return in_maps


# all evacs on Act, in (pass, chunk) order
def _act_idx(pa, c):
    return NCHUNK * pa + c + 1


def _evac_wait(eng, pa, c, sact, sdve):
    eng.wait_ge(sact, _act_idx(pa, c))


# halo geometry: chunk c gets rows [0,16) from chunk c-1 and [112,128) from
# chunk c+1.  src partition start within the neighbour's window:
def _halo_src_lo(c):     # for my rows [0,16): neighbour c-1 partitions
    return STARTS[c] - STARTS[c - 1]                  # 96 or 64
def _halo_src_hi(c):     # for my rows [112,128): neighbour c+1 partitions
    return 112 - (STARTS[c + 1] - STARTS[c])          # 16 or 48


def _build_program():
    import concourse.bass as bass
    from concourse import mybir

    f32 = mybir.dt.float32
    bf16 = mybir.dt.bfloat16
    fp8 = mybir.dt.float8e4
    DR = mybir.MatmulPerfMode.DoubleRow
    nc = bass.Bass("TRN2", target_bir_lowering=False)

    d0b_d = nc.dram_tensor("d0b", [SLAB, CW], bf16, kind="ExternalInput")
    wcs_d = nc.dram_tensor("wcs", [128, N_WC * 128], bf16, kind="ExternalInput")
    wps_d = nc.dram_tensor("wps", [128, N_WP * 256], fp8, kind="ExternalInput")
    xo = nc.dram_tensor("xo", [SH, N_IMG], bf16, kind="ExternalOutput")

    # Dt0: D0 for pass 1, then D3 (pass-2 output); Dt1: D2; Ht: H0 then H2.
    Dt0 = [nc.alloc_sbuf_tensor(f"d0_{c}", [128, CW], bf16) for c in range(NCHUNK)]
    Dt1 = [nc.alloc_sbuf_tensor(f"d1_{c}", [128, CW], bf16) for c in range(NCHUNK)]
    Ht = [nc.alloc_sbuf_tensor(f"h_{c}", [128, HCW], fp8) for c in range(NCHUNK)]
    wcs = nc.alloc_sbuf_tensor("wcss", [128, N_WC * 128], bf16)
    wps = nc.alloc_sbuf_tensor("wpss", [128, N_WP * 256], fp8)
    warmr = nc.alloc_sbuf_tensor("warmr", [128, 528], bf16)
    ps = [nc.alloc_psum_tensor(f"ps{i}", [128, N_IMG], f32) for i in range(2)]

    def hpad_ap(t):
        return bass.AP(t, 0, [[HCW, 128], [HCW - 4, 2], [1, 4]])

    from contextlib import ExitStack
    with ExitStack() as stack:
        block = stack.enter_context(nc.Block())
        sem = lambda n: stack.enter_context(nc.semaphore(n))
        sldw, spe, sact, sdve, spool, sout, swarm = (
            sem("sldw"), sem("spe"), sem("sact"), sem("sdve"), sem("spool"),
            sem("sout"), sem("swarm"))
        sldc = [sem(f"sldc{c}") for c in range(NCHUNK)]
        sldh = [sem(f"sldh{c}") for c in range(NCHUNK)]
        shf = [sem(f"shf{c}") for c in range(NCHUNK)]

        @block.sync
        def _(sp):
            def load_chunk(c):
                sp.dma_start(out=Dt0[c][:, :],
                             in_=d0b_d[STARTS[c]:STARTS[c] + 128, :]
                             ).then_inc(sldc[c], 16)
            load_chunk(0)
            sp.dma_start(out=wcs[:, :], in_=wcs_d[:, :]).then_inc(sldw, 16)
            sp.dma_start(out=wps[:, :], in_=wps_d[:, :]).then_inc(sldw, 16)
            for c in range(1, NCHUNK):
                load_chunk(c)
            # halo fills between pass 1 and pass 2 (D2 + H2 strips)
            for c in range(NCHUNK):
                cm = min(c + 1, NCHUNK - 1)
                sp.wait_ge(sact, _act_idx(0, cm))
                sp.wait_ge(spool, cm + 1)
                if c > 0:
                    s = _halo_src_lo(c)
                    sp.dma_start(out=Dt1[c][0:16, :],
                                 in_=Dt1[c - 1][s:s + 16, :]).then_inc(shf[c], 16)
                    sp.dma_start(out=Ht[c][0:16, :],
                                 in_=Ht[c - 1][s:s + 16, :]).then_inc(shf[c], 16)
                if c < NCHUNK - 1:
                    s = _halo_src_hi(c)
                    sp.dma_start(out=Dt1[c][112:128, :],
                                 in_=Dt1[c + 1][s:s + 16, :]).then_inc(shf[c], 16)
                    sp.dma_start(out=Ht[c][112:128, :],
                                 in_=Ht[c + 1][s:s + 16, :]).then_inc(shf[c], 16)
            # output DMAs
            for c in range(NCHUNK):
                o, p0, p1 = OUT_ROWS[c]
                _evac_wait(sp, 1, c, sact, sdve)
                sp.dma_start(out=xo[o:o + (p1 - p0), :],
                             in_=Dt0[c][p0:p1, 2:2 + N_IMG]).then_inc(sout, 16)
            sp.wait_ge(sout, 16 * NCHUNK)

        @block.tensor
        def _(pe):
            pe.wait_ge(swarm, 1)
            for i in range(12):
                pe.matmul(ps[0][:, 0:512], lhsT=warmr[:, 0:128],
                          rhs=warmr[:, 0:512], start=True, stop=True,
                          skip_group_check=True)
            for pa in range(2):
                Din = Dt0 if pa == 0 else Dt1
                for c in range(NCHUNK):
                    g = NCHUNK * pa + c
                    if pa == 0:
                        if c == 0:
                            pe.wait_ge(sldw, 32)
                        pe.wait_ge(sldh[c], 1)
                    else:
                        pe.wait_ge(shf[c], 16 * 2 * NFILL[c])
                    if g >= 2:
                        _evac_wait(pe, (g - 2) // NCHUNK, (g - 2) % NCHUNK,
                                   sact, sdve)
                    v = {0: 0, NCHUNK - 1: 2}.get(c, 1)
                    wc_i = pa * 3 + v
                    for nt in range(4):
                        o = nt * 512
                        pe.matmul(ps[g % 2][:, o:o + 512],
                                  lhsT=wcs[:, wc_i * 128:(wc_i + 1) * 128],
                                  rhs=Din[c][:, o + 2:o + 514],
                                  start=True, stop=False,
                                  skip_group_check=True)
                        for pi, (dxL, dxR) in enumerate(PAIRS):
                            wp_i = pa * 6 + v * 2 + pi
                            mm = pe.matmul(
                                ps[g % 2][:, o:o + 512],
                                lhsT=bass.AP(wps, wp_i * 256,
                                             [[N_WP * 256, 128], [128, 2], [1, 128]]),
                                rhs=bass.AP(Ht[c], o + dxL + 2,
                                            [[HCW, 128], [dxR - dxL, 2], [1, 512]]),
                                start=False, stop=(pi == len(PAIRS) - 1),
                                perf_mode=DR, skip_group_check=True)
                        if nt == 3:
                            mm.then_inc(spe, 1)

        @block.scalar
        def _(act):
            act.copy(out=warmr[:, 516:528], in_=warmr[:, 0:12])  # act table
            for pa in range(2):
                Dout = Dt1 if pa == 0 else Dt0
                for c in range(NCHUNK):
                    g = NCHUNK * pa + c
                    act.wait_ge(spe, g + 1)
                    act.copy(out=Dout[c][:, 2:2 + N_IMG],
                             in_=ps[g % 2][:, :]).then_inc(sact, 1)

        @block.vector
        def _(dve):
            dve.memset(warmr[:, 0:516], 0.0).then_inc(swarm, 1)
            for c in range(NCHUNK):
                dve.memset(hpad_ap(Ht[c]), 0.0)
            for c in range(NCHUNK):
                dve.wait_ge(sldc[c], 16)
                dve.tensor_copy(out=Ht[c][:, 4:4 + N_IMG],
                                in_=Dt0[c][:, 2:2 + N_IMG]).then_inc(sldh[c], 1)
            # H2 casts (into Ht), each gated on its Act evac
            for c in range(NCHUNK):
                dve.wait_ge(sact, _act_idx(0, c))
                dve.tensor_copy(out=Ht[c][:, 4:4 + N_IMG],
                                in_=Dt1[c][:, 2:2 + N_IMG]).then_inc(spool, 1)
    return nc


def _edge_strips(D0, kern, N):
    """Exact D3 on the 4 left / 4 right edge columns (f64 host compute)."""
    outs = []
    for side in range(2):
        W = 10
        if side == 0:
            s = D0[:, :, 0:W].astype(np.float64)
            colof = 0
        else:
            s = D0[:, :, N_IMG - W:].astype(np.float64)
            colof = N_IMG - W
        for a in range(3):
            sp = np.pad(s, ((0, 0), (8, 8), (2, 2)))
            B = np.zeros_like(s)
            for t in range(KH):
                for dx in range(5):
                    w = kern[a, t, dx]
                    if w != 0.0:
                        B += w * sp[:, t:t + M_IMG, dx:dx + W]
            Ncols = N[a, :, colof:colof + W]
            s = s - B / Ncols[None]
        outs.append(s[:, :, 0:4] if side == 0 else s[:, :, W - 4:])
    return outs[0], outs[1]


_LAST = None  # BassKernelResults of the most recent run (for test harness)


def kernel(X, y, kernel, N_norm):
    global _LAST
    from concourse.bass_utils import run_bass_kernel_spmd

    kern = np.asarray(kernel, np.float64)[:, 0]
    N = np.asarray(N_norm, np.float64)[:, 0]
    in_maps = _host_prep(X, y, kernel, N_norm)
    nc = _build_program()
    res = run_bass_kernel_spmd(nc, in_maps, list(range(BATCH * H_SHARDS)))
    _LAST = res

    yf = np.asarray(y)[:, 0].astype(np.float64)
    D3 = np.empty((BATCH, M_IMG, N_IMG), np.float64)
    for core in range(BATCH * H_SHARDS):
        b, h = core // H_SHARDS, core % H_SHARDS
        D3[b, SH * h:SH * h + SH, :] = res.results[core]["xo"].astype(np.float64)

    D0 = (np.asarray(y) - np.asarray(X))[:, 0]
    left, right = _edge_strips(D0, kern, N)
    D3[:, :, 0:4] = left
    D3[:, :, N_IMG - 4:] = right
    out = (yf - D3).astype(np.float32)
    return out[:, None]
